# revision 15
# baseline (speedup 1.0000x reference)
"""Causal multi-head self-attention (B=8, S=2048, D=384, H=4, Hd=96) on 8
Trainium2 NeuronCores.

Sharding: data-parallel over batch — each core processes one batch element,
weights replicated. No collectives needed.

Per-core algorithm (flash-style, fully SBUF-resident; all matmul inputs
bf16, PSUM accumulation fp32):
  - host passes x[b] pre-transposed as xt [384, 2048] bf16
  - QT/KT computed per head, per 512-col chunk, in [96, S] layout; the
    PSUM->SBUF move is a DVE tensor_scalar that folds in the bias
  - V' natural layout [S, 97*4]: per head [ones_col | V_h]; the ones column
    and bias come from a broadcast [128, 388] tile added on the DVE during
    the PSUM->SBUF move
  - scoresT[k, q] = KT_h^T @ QT_h per 128-row k-tile and 512-col q-chunk,
    left-trimmed to the causal hull; exp on ScalarE (scale=1/sqrt(Hd)
    folded); diagonal 128x128 blocks masked post-exp on the DVE (bf16 2x)
  - PV into PSUM acc [97, 512]: row 0 = softmax denominator (ones-first
    V'), rows 1..96 = unnormalized head output; reciprocal directly from
    PSUM on the DVE custom op, partition_broadcast on GpSimd, normalize on
    DVE (bf16 out)
  - out projection per 128-row tile: 4 matmuls (lhsT = normalized tiles at
    partition offset 1), bias added by the DVE during the PSUM->SBUF move,
    DMA to HBM
  - emission is software-pipelined: scores/exp of group g+1 are emitted
    before PV of group g (pending-FIFO), per-chunk projections and the
    previous chunk's out-projection backfill PE gaps
"""

import sys

sys.path.insert(0, "/opt/trn_rl_repo")

import ml_dtypes
import numpy as np

import concourse.bass as bass
import concourse.tile as tile
from concourse import bacc, mybir
from concourse.bass_utils import run_bass_kernel_spmd

N_CORES = 8
S = 2048
D = 384
H = 4
HD = 96
CH = 512          # q-chunk width
NCH = S // CH     # 4 q-chunks
P = 128           # k-tile height / partition dim
KTN = S // P      # 16 k-tiles
SCALE = 1.0 / np.sqrt(HD)
VW = 128          # V' cols per head: [ones | 31*0 | V_h] (padded so
VOFF = 32         # out-proj lhsT can start at base partition 32)

F32 = mybir.dt.float32
BF16 = mybir.dt.bfloat16


def build_nc(repeat=1, variant=(), loop_n=0):
    nc = bacc.Bacc("TRN2", target_bir_lowering=False, debug=False,
                   enable_asserts=False, num_devices=N_CORES)

    xt_d = nc.dram_tensor("xt", [D, S], BF16, kind="ExternalInput").ap()
    wq_d = nc.dram_tensor("wq", [D, D], BF16, kind="ExternalInput").ap()
    wk_d = nc.dram_tensor("wk", [D, D], BF16, kind="ExternalInput").ap()
    wv_d = nc.dram_tensor("wv", [D, VW * H], BF16, kind="ExternalInput").ap()
    wo_d = nc.dram_tensor("wo", [VW * H, D], BF16, kind="ExternalInput").ap()
    bqh_d = nc.dram_tensor("bqh", [HD, H], F32, kind="ExternalInput").ap()
    bkh_d = nc.dram_tensor("bkh", [HD, H], F32, kind="ExternalInput").ap()
    vb_d = nc.dram_tensor("vb", [1, VW * H], F32, kind="ExternalInput").ap()
    bo_d = nc.dram_tensor("bo", [1, D], F32, kind="ExternalInput").ap()
    msk_d = nc.dram_tensor("msk", [P, P], BF16, kind="ExternalInput").ap()
    out_d = nc.dram_tensor("out", [S, D], F32, kind="ExternalOutput").ap()

    Exp = mybir.ActivationFunctionType.Exp
    mult = mybir.AluOpType.mult
    add = mybir.AluOpType.add

    with tile.TileContext(nc) as tc:
        wpool = tc.alloc_tile_pool(name="w", bufs=1)
        xpool = tc.alloc_tile_pool(name="x", bufs=1)
        qkt_pool = tc.alloc_tile_pool(name="qkt", bufs=1)
        vpool = tc.alloc_tile_pool(name="v", bufs=1)
        ppool = tc.alloc_tile_pool(name="p", bufs=4)
        onpool = tc.alloc_tile_pool(name="on", bufs=1)
        rpool = tc.alloc_tile_pool(name="r", bufs=3)
        fspool = tc.alloc_tile_pool(name="fs", bufs=3)
        qkpool = tc.alloc_tile_pool(name="qkps", bufs=2, space="PSUM")
        accpool = tc.alloc_tile_pool(name="accps", bufs=2, space="PSUM")
        pspool = tc.alloc_tile_pool(name="pps", bufs=2, space="PSUM")

        import contextlib
        loop_ctx = (tc.For_i(0, loop_n, 1) if loop_n
                    else contextlib.nullcontext())
        with loop_ctx:
          for _rep in range(repeat):
            # ---- loads split across issue queues so transfer of the
            # first-needed tensors (xt+wq+wk, on SP) isn't stuck behind
            # issue overhead of the rest (Act queue: wv/wo; Pool queue:
            # small constants) ----
            xt_sb, wq_sb, wk_sb, wv_sb, wo_sb = [], [], [], [], []
            for t in range(3):
                xt = xpool.tile([P, S], BF16, name=f"xt{t}", tag=f"xt{t}")
                xt_sb.append(xt)
                wq_sb.append(wpool.tile([P, D], BF16, name=f"wq{t}",
                                        tag=f"wq{t}"))
                wk_sb.append(wpool.tile([P, D], BF16, name=f"wk{t}",
                                        tag=f"wk{t}"))
                wv_sb.append(wpool.tile([P, VW * H], BF16, name=f"wv{t}",
                                        tag=f"wv{t}"))
            # xt is chunked along q so the first projections start after
            # ~384KB instead of the full 1.5MB
            for t in range(3):
                nc.sync.dma_start(xt_sb[t][:, 0:CH],
                                  xt_d[P * t:P * t + P, 0:CH])
                nc.gpsimd.dma_start(wq_sb[t][:], wq_d[P * t:P * t + P, :])
                nc.scalar.dma_start(wk_sb[t][:], wk_d[P * t:P * t + P, :])
            for t in range(3):
                nc.sync.dma_start(xt_sb[t][:, CH:S],
                                  xt_d[P * t:P * t + P, CH:S])
            for t in range(3):
                nc.scalar.dma_start(wv_sb[t][:], wv_d[P * t:P * t + P, :])
            for h in range(H):
                # rows 0..31 are zeros (host-padded) so the full-128
                # contraction in the out projection ignores the
                # denominator/pad rows of the on tiles
                wot = wpool.tile([P, D], BF16, name=f"wo{h}", tag=f"wo{h}")
                nc.scalar.dma_start(wot[:], wo_d[VW * h:VW * h + P, :])
                wo_sb.append(wot)

            msk_sb = wpool.tile([P, P], BF16, name="msk", tag="msk")
            nc.gpsimd.dma_start(msk_sb[:], msk_d[:, :])
            bq_sb = wpool.tile([HD, H], F32, name="bq", tag="bq")
            nc.gpsimd.dma_start(bq_sb[:], bqh_d[:, :])
            bk_sb = wpool.tile([HD, H], F32, name="bk", tag="bk")
            nc.gpsimd.dma_start(bk_sb[:], bkh_d[:, :])
            vb_sb = wpool.tile([1, VW * H], F32, name="vb", tag="vb")
            nc.gpsimd.dma_start(vb_sb[:], vb_d[:, :])
            bo_sb = wpool.tile([1, D], F32, name="bo", tag="bo")
            nc.gpsimd.dma_start(bo_sb[:], bo_d[:, :])

            # warmup: force the Exp act-table load off the critical path
            warm = wpool.tile([1, 1], BF16, name="warm", tag="warm")
            nc.scalar.activation(warm[:], msk_sb[0:1, 0:1], Exp)

            # broadcast V'/out biases across partitions once (GpSimd)
            vb_bc = wpool.tile([P, VW * H], F32, name="vbbc", tag="vbbc")
            nc.gpsimd.partition_broadcast(vb_bc[:], vb_sb[:], channels=P)
            bo_bc = wpool.tile([P, D], F32, name="bobc", tag="bobc")
            nc.gpsimd.partition_broadcast(bo_bc[:], bo_sb[:], channels=P)

            # persistent SBUF results
            qt_sb = [qkt_pool.tile([HD, S], BF16, name=f"qt{h}", tag=f"qt{h}")
                     for h in range(H)]
            kt_sb = [qkt_pool.tile([HD, S], BF16, name=f"kt{h}", tag=f"kt{h}")
                     for h in range(H)]
            v_sb = [vpool.tile([P, VW * H], BF16, name=f"v{st}", tag=f"v{st}")
                    for st in range(KTN)]
            on_sb = {}
            for ci in range(NCH):
                for h in range(H):
                    on_sb[(ci, h)] = onpool.tile(
                        [VW, CH], BF16, name=f"on{ci}_{h}",
                        tag=f"on{ci}_{h}")

            pending = []   # pv closures, popped one per emitted group
            bfq = []       # backfill: next chunk's projection pieces
            opq = []       # out-projections, deferred into the last chunk
                           # where no projection backfill remains

            def pump():
                if pending:
                    pending.pop(0)()

            def pop_bf(allow_op=False):
                if bfq:
                    bfq.pop(0)()
                elif allow_op and opq:
                    opq.pop(0)()

            def emit_qk_proj(h, ci, only=None):
                specs = ((wq_sb, bq_sb, qt_sb), (wk_sb, bk_sb, kt_sb))
                if only is not None:
                    specs = (specs[only],)
                for w_sb, b_sb, dst in specs:
                    ps = pspool.tile([HD, CH], F32, name="projps", tag="pps")
                    for t in range(3):
                        nc.tensor.matmul(
                            ps[:],
                            w_sb[t][:, HD * h:HD * h + HD],
                            xt_sb[t][:, CH * ci:CH * ci + CH],
                            start=(t == 0), stop=(t == 2))
                    nc.vector.tensor_scalar(
                        dst[h][:, CH * ci:CH * ci + CH], ps[:],
                        b_sb[:, h:h + 1], None, add)

            def emit_v_proj(st):
                ps = pspool.tile([P, VW * H], F32, name="vps", tag="pps")
                for t in range(3):
                    nc.tensor.matmul(ps[:], xt_sb[t][:, P * st:P * st + P],
                                     wv_sb[t][:], start=(t == 0), stop=(t == 2))
                nc.vector.tensor_tensor(v_sb[st][:], ps[:], vb_bc[:], op=add)

            def emit_out_proj(ci, sj):
                st = 4 * ci + sj
                fo = pspool.tile([P, D], F32, name="fo", tag="pps")
                for h in range(H):
                    nc.tensor.matmul(
                        fo[:], on_sb[(ci, h)][:, P * sj:P * sj + P],
                        wo_sb[h][:], start=(h == 0), stop=(h == 3))
                fs = fspool.tile([P, D], F32, name="fs", tag="fs")
                nc.vector.tensor_tensor(fs[:], fo[:], bo_bc[:], op=add)
                nc.sync.dma_start(out_d[P * st:P * st + P, :], fs[:])

            def make_pv(ci, h, kts, qk_rts, pt, nkt):
                acc = pv_acc[(ci, h)]

                def pv():
                    for j, kt in enumerate(kts):
                        rt = P * kt - CH * ci
                        scol = max(rt, 0)
                        nc.tensor.matmul(
                            acc[0:VW, scol:CH],
                            v_sb[kt][:, VW * h:VW * h + VW],
                            pt[:, CH * j + scol:CH * (j + 1)],
                            start=(kt == 0), stop=(kt == nkt - 1),
                            skip_group_check=True)
                    if kts[-1] == nkt - 1:
                        # chunk-head finished: normalize
                        den0 = rpool.tile([1, CH], F32, name="den0", tag="den0")
                        nc.vector.reciprocal_approx_fast(
                            out=den0[:], in_=acc[0:1, :])
                        rb = rpool.tile([VW, CH], F32, name="rb", tag="rb")
                        nc.gpsimd.partition_broadcast(rb[:], den0[:],
                                                      channels=VW)
                        nc.vector.tensor_tensor(
                            on_sb[(ci, h)][:], acc[0:VW, :], rb[:], op=mult)
                        if h == H - 1:
                            for sj in range(4):
                                opq.append(
                                    lambda ci=ci, sj=sj: emit_out_proj(ci, sj))
                return pv

            # warm the PE p-state ramp during the initial DMA wait
            wscr = wpool.tile([1, CH], BF16, name="wscr", tag="wscr")
            nc.vector.memset(wscr[:], 0)
            for _ in range(4):
                wps = pspool.tile([1, P], F32, name="wps", tag="pps")
                nc.tensor.matmul(wps[:], wscr[:, 0:1], wscr[:, 0:P],
                                 start=True, stop=True)

            def emit_projs(ci):
                if ci == 0:
                    # Q projections first: they only need wq + the first xt
                    # chunk, which arrive before wk
                    for h in range(H):
                        emit_qk_proj(h, 0, only=0)
                    for h in range(H):
                        emit_qk_proj(h, 0, only=1)
                else:
                    for h in range(H):
                        emit_qk_proj(h, ci)
                for sj in range(4):
                    emit_v_proj(4 * ci + sj)

            pv_acc = {}
            emit_projs(0)
            for ci in range(NCH):
                if ci + 1 < NCH:
                    for hh in range(H):
                        bfq.append(lambda hh=hh, ci=ci: emit_qk_proj(hh, ci + 1))
                    for sj in range(4):
                        bfq.append(lambda st=4 * (ci + 1) + sj: emit_v_proj(st))
                for h in range(H):
                    pop_bf(allow_op=(ci == NCH - 1))
                    nkt = 4 * (ci + 1)
                    acc = accpool.tile([VW, CH], F32, name="acc", tag="acc")
                    pv_acc[(ci, h)] = acc
                    for g0 in range(0, nkt, 2):
                        kts = [g0, g0 + 1]
                        qk = qkpool.tile([P, 2 * CH], F32, name="qk", tag="qk")
                        pt = ppool.tile([P, 2 * CH], BF16, name="pt", tag="pt")
                        rts = []
                        for j, kt in enumerate(kts):
                            rt = P * kt - CH * ci
                            rts.append(rt)
                            scol = max(rt, 0)
                            nc.tensor.matmul(
                                qk[:, CH * j + scol:CH * (j + 1)],
                                kt_sb[h][:, P * kt:P * kt + P],
                                qt_sb[h][:, CH * ci + scol:CH * ci + CH],
                                start=True, stop=True)
                        if rts[0] >= 0:
                            # diagonal pair: per-tile trimmed exp + mask
                            for j, rt in enumerate(rts):
                                nc.scalar.activation(
                                    pt[:, CH * j + rt:CH * (j + 1)],
                                    qk[:, CH * j + rt:CH * (j + 1)],
                                    Exp, scale=float(SCALE))
                                nc.vector.tensor_tensor(
                                    pt[:, CH * j + rt:CH * j + rt + P],
                                    pt[:, CH * j + rt:CH * j + rt + P],
                                    msk_sb[:], op=mult)
                        else:
                            nc.scalar.activation(
                                pt[:], qk[:], Exp, scale=float(SCALE))
                        if (g0 // 2) % 2 == 1:
                            pop_bf(allow_op=(ci == NCH - 1))
                        pump()
                        pending.append(make_pv(ci, h, kts, rts, pt, nkt))
                while bfq and h == H - 1:
                    pop_bf()
            while pending or bfq or opq:
                pump()
                pop_bf(allow_op=True)

        for pool in (pspool, accpool, qkpool, fspool, rpool, onpool, ppool,
                     vpool, qkt_pool, xpool, wpool):
            pool.release()

    nc.finalize()
    return nc


_NC_CACHE = None


def get_nc():
    global _NC_CACHE
    if _NC_CACHE is None:
        _NC_CACHE = build_nc()
    return _NC_CACHE


def host_prep(x, Wq, bq, Wk, bk, Wv, bv, Wo, bo):
    """Build per-core input maps (layout/dtype prep only)."""
    bf = ml_dtypes.bfloat16
    x = np.asarray(x, dtype=np.float32)
    Wq = np.asarray(Wq, dtype=np.float32).astype(bf)
    Wk = np.asarray(Wk, dtype=np.float32).astype(bf)
    Wv = np.asarray(Wv, dtype=np.float32)
    Wo = np.asarray(Wo, dtype=np.float32)
    bq = np.asarray(bq, dtype=np.float32)
    bk = np.asarray(bk, dtype=np.float32)
    bv = np.asarray(bv, dtype=np.float32)
    bo = np.asarray(bo, dtype=np.float32)

    wo_pad = np.zeros((VW * H, D), np.float32)
    for h in range(H):
        wo_pad[VW * h + VOFF:VW * h + VW] = Wo[HD * h:HD * h + HD]

    wv_x = np.zeros((D, VW * H), np.float32)
    vbias = np.zeros((1, VW * H), np.float32)
    for h in range(H):
        wv_x[:, VW * h + VOFF:VW * h + VW] = Wv[:, HD * h:HD * h + HD]
        vbias[0, VW * h] = 1.0
        vbias[0, VW * h + VOFF:VW * h + VW] = bv[HD * h:HD * h + HD]

    jj = np.arange(P)[None, :]
    pp = np.arange(P)[:, None]
    msk = (jj >= pp).astype(bf)

    bqh = np.ascontiguousarray(bq.reshape(H, HD).T)
    bkh = np.ascontiguousarray(bk.reshape(H, HD).T)
    common = dict(wq=np.ascontiguousarray(Wq), wk=np.ascontiguousarray(Wk),
                  wv=wv_x.astype(bf), wo=wo_pad.astype(bf),
                  bqh=bqh, bkh=bkh, vb=vbias,
                  bo=np.ascontiguousarray(bo.reshape(1, D)), msk=msk)
    return [dict(xt=np.ascontiguousarray(x[b].T.astype(bf)), **common)
            for b in range(x.shape[0])]


def kernel(**inputs):
    in_maps = host_prep(**inputs)
    nc = get_nc()
    res = run_bass_kernel_spmd(nc, in_maps, core_ids=list(range(N_CORES)))
    return np.stack([res.results[b]["out"] for b in range(N_CORES)], axis=0)


# revision 17
# speedup vs baseline: 1.0105x; 1.0105x over previous
"""Causal multi-head self-attention (B=8, S=2048, D=384, H=4, Hd=96) on 8
Trainium2 NeuronCores.

Sharding: data-parallel over batch — each core processes one batch element,
weights replicated. No collectives needed.

Per-core algorithm (flash-style, fully SBUF-resident; all matmul inputs
bf16, PSUM accumulation fp32):
  - host passes x[b] pre-transposed as xt [384, 2048] bf16
  - QT/KT computed per head, per 512-col chunk, in [96, S] layout; the
    PSUM->SBUF move is a DVE tensor_scalar that folds in the bias
  - V' natural layout [S, 97*4]: per head [ones_col | V_h]; the ones column
    and bias come from a broadcast [128, 388] tile added on the DVE during
    the PSUM->SBUF move
  - scoresT[k, q] = KT_h^T @ QT_h per 128-row k-tile and 512-col q-chunk,
    left-trimmed to the causal hull; exp on ScalarE (scale=1/sqrt(Hd)
    folded); diagonal 128x128 blocks masked post-exp on the DVE (bf16 2x)
  - PV into PSUM acc [97, 512]: row 0 = softmax denominator (ones-first
    V'), rows 1..96 = unnormalized head output; reciprocal directly from
    PSUM on the DVE custom op, partition_broadcast on GpSimd, normalize on
    DVE (bf16 out)
  - out projection per 128-row tile: 4 matmuls (lhsT = normalized tiles at
    partition offset 1), bias added by the DVE during the PSUM->SBUF move,
    DMA to HBM
  - emission is software-pipelined: scores/exp of group g+1 are emitted
    before PV of group g (pending-FIFO), per-chunk projections and the
    previous chunk's out-projection backfill PE gaps
"""

import os
import sys

sys.path.insert(0, "/opt/trn_rl_repo")

import ml_dtypes
import numpy as np

import concourse.bass as bass
import concourse.tile as tile
from concourse import bacc, mybir
from concourse.bass_utils import run_bass_kernel_spmd

N_CORES = 8
S = 2048
D = 384
H = 4
HD = 96
CH = 512          # q-chunk width
NCH = S // CH     # 4 q-chunks
P = 128           # k-tile height / partition dim
KTN = S // P      # 16 k-tiles
SCALE = 1.0 / np.sqrt(HD)
VW = 128          # V' cols per head: [ones | 31*0 | V_h] (padded so
VOFF = 32         # out-proj lhsT can start at base partition 32)

F32 = mybir.dt.float32
BF16 = mybir.dt.bfloat16


def build_nc(repeat=1, variant=None, loop_n=0):
    if variant is None:
        variant = tuple(v for v in os.environ.get("ATTN_VARIANT", "").split(",")
                        if v)
    nc = bacc.Bacc("TRN2", target_bir_lowering=False, debug=False,
                   enable_asserts=False, num_devices=N_CORES)

    xt_d = nc.dram_tensor("xt", [D, S], BF16, kind="ExternalInput").ap()
    wq_d = nc.dram_tensor("wq", [D, D], BF16, kind="ExternalInput").ap()
    wk_d = nc.dram_tensor("wk", [D, D], BF16, kind="ExternalInput").ap()
    wv_d = nc.dram_tensor("wv", [D, VW * H], BF16, kind="ExternalInput").ap()
    wo_d = nc.dram_tensor("wo", [VW * H, D], BF16, kind="ExternalInput").ap()
    bqh_d = nc.dram_tensor("bqh", [HD, H], F32, kind="ExternalInput").ap()
    bkh_d = nc.dram_tensor("bkh", [HD, H], F32, kind="ExternalInput").ap()
    vb_d = nc.dram_tensor("vb", [1, VW * H], F32, kind="ExternalInput").ap()
    bo_d = nc.dram_tensor("bo", [1, D], F32, kind="ExternalInput").ap()
    msk_d = nc.dram_tensor("msk", [P, P], BF16, kind="ExternalInput").ap()
    out_d = nc.dram_tensor("out", [S, D], F32, kind="ExternalOutput").ap()

    Exp = mybir.ActivationFunctionType.Exp
    mult = mybir.AluOpType.mult
    add = mybir.AluOpType.add

    with tile.TileContext(nc) as tc:
        wpool = tc.alloc_tile_pool(name="w", bufs=1)
        xpool = tc.alloc_tile_pool(name="x", bufs=1)
        qkt_pool = tc.alloc_tile_pool(name="qkt", bufs=1)
        vpool = tc.alloc_tile_pool(name="v", bufs=1)
        ppool = tc.alloc_tile_pool(name="p", bufs=4)
        onpool = tc.alloc_tile_pool(name="on", bufs=1)
        rpool = tc.alloc_tile_pool(name="r", bufs=3)
        fspool = tc.alloc_tile_pool(name="fs", bufs=3)
        qkpool = tc.alloc_tile_pool(name="qkps", bufs=2, space="PSUM")
        accpool = tc.alloc_tile_pool(name="accps", bufs=2, space="PSUM")
        pspool = tc.alloc_tile_pool(name="pps", bufs=2, space="PSUM")

        import contextlib
        loop_ctx = (tc.For_i(0, loop_n, 1) if loop_n
                    else contextlib.nullcontext())
        with loop_ctx:
          for _rep in range(repeat):
            # ---- loads split across issue queues so transfer of the
            # first-needed tensors (xt+wq+wk, on SP) isn't stuck behind
            # issue overhead of the rest (Act queue: wv/wo; Pool queue:
            # small constants) ----
            xt_sb, wq_sb, wk_sb, wv_sb, wo_sb = [], [], [], [], []
            for t in range(3):
                xt = xpool.tile([P, S], BF16, name=f"xt{t}", tag=f"xt{t}")
                xt_sb.append(xt)
                wq_sb.append(wpool.tile([P, D], BF16, name=f"wq{t}",
                                        tag=f"wq{t}"))
                wk_sb.append(wpool.tile([P, D], BF16, name=f"wk{t}",
                                        tag=f"wk{t}"))
                wv_sb.append(wpool.tile([P, VW * H], BF16, name=f"wv{t}",
                                        tag=f"wv{t}"))
            # xt is chunked along q so the first projections start after
            # ~384KB instead of the full 1.5MB
            ldq = nc.sync if "legacy_dma" in variant else None
            if ldq is not None:
                for t in range(3):
                    ldq.dma_start(wq_sb[t][:], wq_d[P * t:P * t + P, :])
                    ldq.dma_start(xt_sb[t][:], xt_d[P * t:P * t + P, :])
                    ldq.dma_start(wk_sb[t][:], wk_d[P * t:P * t + P, :])
                for t in range(3):
                    ldq.dma_start(wv_sb[t][:], wv_d[P * t:P * t + P, :])
            else:
                for t in range(3):
                    nc.sync.dma_start(xt_sb[t][:, 0:CH],
                                      xt_d[P * t:P * t + P, 0:CH])
                    nc.gpsimd.dma_start(wq_sb[t][:], wq_d[P * t:P * t + P, :])
                    nc.scalar.dma_start(wk_sb[t][:], wk_d[P * t:P * t + P, :])
                for t in range(3):
                    nc.sync.dma_start(xt_sb[t][:, CH:S],
                                      xt_d[P * t:P * t + P, CH:S])
                for t in range(3):
                    nc.scalar.dma_start(wv_sb[t][:], wv_d[P * t:P * t + P, :])
            for h in range(H):
                # rows 0..31 are zeros (host-padded) so the full-128
                # contraction in the out projection ignores the
                # denominator/pad rows of the on tiles
                wot = wpool.tile([P, D], BF16, name=f"wo{h}", tag=f"wo{h}")
                (ldq or nc.scalar).dma_start(wot[:], wo_d[VW * h:VW * h + P, :])
                wo_sb.append(wot)

            msk_sb = wpool.tile([P, P], BF16, name="msk", tag="msk")
            (ldq or nc.gpsimd).dma_start(msk_sb[:], msk_d[:, :])
            bq_sb = wpool.tile([HD, H], F32, name="bq", tag="bq")
            (ldq or nc.gpsimd).dma_start(bq_sb[:], bqh_d[:, :])
            bk_sb = wpool.tile([HD, H], F32, name="bk", tag="bk")
            (ldq or nc.gpsimd).dma_start(bk_sb[:], bkh_d[:, :])
            vb_sb = wpool.tile([1, VW * H], F32, name="vb", tag="vb")
            (ldq or nc.gpsimd).dma_start(vb_sb[:], vb_d[:, :])
            bo_sb = wpool.tile([1, D], F32, name="bo", tag="bo")
            (ldq or nc.gpsimd).dma_start(bo_sb[:], bo_d[:, :])

            # warmup: force the Exp act-table load off the critical path
            warm = wpool.tile([1, 1], BF16, name="warm", tag="warm")
            nc.scalar.activation(warm[:], msk_sb[0:1, 0:1], Exp)

            # broadcast V'/out biases across partitions once (GpSimd)
            vb_bc = wpool.tile([P, VW * H], F32, name="vbbc", tag="vbbc")
            nc.gpsimd.partition_broadcast(vb_bc[:], vb_sb[:], channels=P)
            bo_bc = wpool.tile([P, D], F32, name="bobc", tag="bobc")
            nc.gpsimd.partition_broadcast(bo_bc[:], bo_sb[:], channels=P)

            # persistent SBUF results
            qt_sb = [qkt_pool.tile([HD, S], BF16, name=f"qt{h}", tag=f"qt{h}")
                     for h in range(H)]
            kt_sb = [qkt_pool.tile([HD, S], BF16, name=f"kt{h}", tag=f"kt{h}")
                     for h in range(H)]
            v_sb = [vpool.tile([P, VW * H], BF16, name=f"v{st}", tag=f"v{st}")
                    for st in range(KTN)]
            on_sb = {}
            for ci in range(NCH):
                for h in range(H):
                    on_sb[(ci, h)] = onpool.tile(
                        [VW, CH], BF16, name=f"on{ci}_{h}",
                        tag=f"on{ci}_{h}")

            pending = []   # pv closures, popped one per emitted group
            bfq = []       # backfill: next chunk's projection pieces
            opq = []       # out-projections, deferred into the last chunk
                           # where no projection backfill remains

            def pump():
                if pending:
                    pending.pop(0)()

            def pop_bf(allow_op=False):
                if bfq:
                    bfq.pop(0)()
                elif allow_op and opq:
                    opq.pop(0)()

            def emit_qk_proj(h, ci, only=None, use_act=False):
                specs = ((wq_sb, bq_sb, qt_sb), (wk_sb, bk_sb, kt_sb))
                if only is not None:
                    specs = (specs[only],)
                for w_sb, b_sb, dst in specs:
                    ps = pspool.tile([HD, CH], F32, name="projps", tag="pps")
                    for t in range(3):
                        nc.tensor.matmul(
                            ps[:],
                            w_sb[t][:, HD * h:HD * h + HD],
                            xt_sb[t][:, CH * ci:CH * ci + CH],
                            start=(t == 0), stop=(t == 2))
                    if use_act:
                        # Act has slack while these chunks' projections are
                        # emitted; DVE is co-saturated there
                        nc.scalar.add(dst[h][:, CH * ci:CH * ci + CH],
                                      ps[:], b_sb[:, h:h + 1])
                    else:
                        nc.vector.tensor_scalar(
                            dst[h][:, CH * ci:CH * ci + CH], ps[:],
                            b_sb[:, h:h + 1], None, add)

            def emit_v_proj(st):
                ps = pspool.tile([P, VW * H], F32, name="vps", tag="pps")
                for t in range(3):
                    nc.tensor.matmul(ps[:], xt_sb[t][:, P * st:P * st + P],
                                     wv_sb[t][:], start=(t == 0), stop=(t == 2))
                nc.vector.tensor_tensor(v_sb[st][:], ps[:], vb_bc[:], op=add)

            def emit_out_proj(ci, sj):
                st = 4 * ci + sj
                fo = pspool.tile([P, D], F32, name="fo", tag="pps")
                for h in range(H):
                    nc.tensor.matmul(
                        fo[:], on_sb[(ci, h)][:, P * sj:P * sj + P],
                        wo_sb[h][:], start=(h == 0), stop=(h == 3))
                fs = fspool.tile([P, D], F32, name="fs", tag="fs")
                nc.vector.tensor_tensor(fs[:], fo[:], bo_bc[:], op=add)
                nc.sync.dma_start(out_d[P * st:P * st + P, :], fs[:])

            def make_pv(ci, h, kts, qk_rts, pt, nkt):
                acc = pv_acc[(ci, h)]

                def pv():
                    for j, kt in enumerate(kts):
                        rt = P * kt - CH * ci
                        scol = max(rt, 0)
                        nc.tensor.matmul(
                            acc[0:VW, scol:CH],
                            v_sb[kt][:, VW * h:VW * h + VW],
                            pt[:, CH * j + scol:CH * (j + 1)],
                            start=(kt == 0), stop=(kt == nkt - 1),
                            skip_group_check=True)
                    if kts[-1] == nkt - 1:
                        # chunk-head finished: normalize
                        den0 = rpool.tile([1, CH], F32, name="den0", tag="den0")
                        nc.vector.reciprocal_approx_fast(
                            out=den0[:], in_=acc[0:1, :])
                        rb = rpool.tile([VW, CH], F32, name="rb", tag="rb")
                        nc.gpsimd.partition_broadcast(rb[:], den0[:],
                                                      channels=VW)
                        nc.vector.tensor_tensor(
                            on_sb[(ci, h)][:], acc[0:VW, :], rb[:], op=mult)
                        if h == H - 1:
                            for sj in range(4):
                                opq.append(
                                    lambda ci=ci, sj=sj: emit_out_proj(ci, sj))
                return pv

            # warm the PE p-state ramp during the initial DMA wait
            wscr = wpool.tile([1, CH], BF16, name="wscr", tag="wscr")
            nc.vector.memset(wscr[:], 0)
            for _ in range(4):
                wps = pspool.tile([1, P], F32, name="wps", tag="pps")
                nc.tensor.matmul(wps[:], wscr[:, 0:1], wscr[:, 0:P],
                                 start=True, stop=True)

            def emit_projs(ci):
                if ci == 0:
                    # Q projections first: they only need wq + the first xt
                    # chunk, which arrive before wk
                    for h in range(H):
                        emit_qk_proj(h, 0, only=0)
                    for h in range(H):
                        emit_qk_proj(h, 0, only=1)
                else:
                    for h in range(H):
                        emit_qk_proj(h, ci)
                for sj in range(4):
                    emit_v_proj(4 * ci + sj)

            pv_acc = {}
            emit_projs(0)
            for ci in range(NCH):
                if ci + 1 < NCH:
                    for hh in range(H):
                        bfq.append(lambda hh=hh, ci=ci: emit_qk_proj(
                            hh, ci + 1, use_act=(ci + 1 < NCH - 1)))
                    for sj in range(4):
                        bfq.append(lambda st=4 * (ci + 1) + sj: emit_v_proj(st))
                for h in range(H):
                    pop_bf(allow_op=(ci == NCH - 1))
                    nkt = 4 * (ci + 1)
                    acc = accpool.tile([VW, CH], F32, name="acc", tag="acc")
                    pv_acc[(ci, h)] = acc
                    for g0 in range(0, nkt, 2):
                        kts = [g0, g0 + 1]
                        qk = qkpool.tile([P, 2 * CH], F32, name="qk", tag="qk")
                        pt = ppool.tile([P, 2 * CH], BF16, name="pt", tag="pt")
                        rts = []
                        for j, kt in enumerate(kts):
                            rt = P * kt - CH * ci
                            rts.append(rt)
                            scol = max(rt, 0)
                            nc.tensor.matmul(
                                qk[:, CH * j + scol:CH * (j + 1)],
                                kt_sb[h][:, P * kt:P * kt + P],
                                qt_sb[h][:, CH * ci + scol:CH * ci + CH],
                                start=True, stop=True)
                        if rts[0] >= 0:
                            # diagonal pair: per-tile trimmed exp + mask
                            for j, rt in enumerate(rts):
                                nc.scalar.activation(
                                    pt[:, CH * j + rt:CH * (j + 1)],
                                    qk[:, CH * j + rt:CH * (j + 1)],
                                    Exp, scale=float(SCALE))
                                nc.vector.tensor_tensor(
                                    pt[:, CH * j + rt:CH * j + rt + P],
                                    pt[:, CH * j + rt:CH * j + rt + P],
                                    msk_sb[:], op=mult)
                        else:
                            nc.scalar.activation(
                                pt[:], qk[:], Exp, scale=float(SCALE))
                        if (g0 // 2) % 2 == 1:
                            pop_bf(allow_op=(ci == NCH - 1))
                        pump()
                        pending.append(make_pv(ci, h, kts, rts, pt, nkt))
                while bfq and h == H - 1:
                    pop_bf()
            while pending or bfq or opq:
                pump()
                pop_bf(allow_op=True)

        for pool in (pspool, accpool, qkpool, fspool, rpool, onpool, ppool,
                     vpool, qkt_pool, xpool, wpool):
            pool.release()

    nc.finalize()
    return nc


_NC_CACHE = None


def get_nc():
    global _NC_CACHE
    if _NC_CACHE is None:
        _NC_CACHE = build_nc()
    return _NC_CACHE


def host_prep(x, Wq, bq, Wk, bk, Wv, bv, Wo, bo):
    """Build per-core input maps (layout/dtype prep only)."""
    bf = ml_dtypes.bfloat16
    x = np.asarray(x, dtype=np.float32)
    Wq = np.asarray(Wq, dtype=np.float32).astype(bf)
    Wk = np.asarray(Wk, dtype=np.float32).astype(bf)
    Wv = np.asarray(Wv, dtype=np.float32)
    Wo = np.asarray(Wo, dtype=np.float32)
    bq = np.asarray(bq, dtype=np.float32)
    bk = np.asarray(bk, dtype=np.float32)
    bv = np.asarray(bv, dtype=np.float32)
    bo = np.asarray(bo, dtype=np.float32)

    wo_pad = np.zeros((VW * H, D), np.float32)
    for h in range(H):
        wo_pad[VW * h + VOFF:VW * h + VW] = Wo[HD * h:HD * h + HD]

    wv_x = np.zeros((D, VW * H), np.float32)
    vbias = np.zeros((1, VW * H), np.float32)
    for h in range(H):
        wv_x[:, VW * h + VOFF:VW * h + VW] = Wv[:, HD * h:HD * h + HD]
        vbias[0, VW * h] = 1.0
        vbias[0, VW * h + VOFF:VW * h + VW] = bv[HD * h:HD * h + HD]

    jj = np.arange(P)[None, :]
    pp = np.arange(P)[:, None]
    msk = (jj >= pp).astype(bf)

    bqh = np.ascontiguousarray(bq.reshape(H, HD).T)
    bkh = np.ascontiguousarray(bk.reshape(H, HD).T)
    common = dict(wq=np.ascontiguousarray(Wq), wk=np.ascontiguousarray(Wk),
                  wv=wv_x.astype(bf), wo=wo_pad.astype(bf),
                  bqh=bqh, bkh=bkh, vb=vbias,
                  bo=np.ascontiguousarray(bo.reshape(1, D)), msk=msk)
    return [dict(xt=np.ascontiguousarray(x[b].T.astype(bf)), **common)
            for b in range(x.shape[0])]


def kernel(**inputs):
    in_maps = host_prep(**inputs)
    nc = get_nc()
    res = run_bass_kernel_spmd(nc, in_maps, core_ids=list(range(N_CORES)))
    return np.stack([res.results[b]["out"] for b in range(N_CORES)], axis=0)
